# revision 1
# baseline (speedup 1.0000x reference)
"""Distributed 2-layer GCN (GCLEncoder) on 8 Trainium2 NeuronCores — Bass/Tile.

kernel(**inputs) takes the FULL inputs (x [100000,128] f32, W1 [128,64],
b1 [64], W2 [64,32], b2 [32], edge_index [2,1600000] i32) and returns the
FULL output z [100000, 32] f32.

Structure (v2 — batched dma_gather + bf16 one-hot scatter):
- Destination nodes sharded contiguously across 8 cores (12500 each, padded
  to 12544 = 98 tiles of 128). The host balances nodes into the 98 one-tile
  groups by per-window degree so per-(group,window) edge counts are even.
- Per-layer node feature tables G = dinv_src * feat, bf16, padded to 128
  features (256B rows = dma_gather granularity), exchanged via AllGather.
  dinv_dst is applied per node AFTER aggregation (the GCN norm factorizes),
  so no per-edge math is needed.
- Self-loops are not gathered: each node's own table row is kept in SBUF
  from the dense stage and added at finish time.
- Edges are bucketed by (dst group, src row window); 4 windows of 25088
  rows keep gather indices in int16. Gathers run as 16 dma_gather
  instructions per layer (quarter of groups x window each).
- Scatter-add into the 128 dsts of a group is a one-hot matmul: S[e,d] =
  (dst_rel[e] == d) built in bf16 alternately on DVE/Pool, then
  matmul(psum += S^T @ rows) over the TBW blocks of a (group,window) run,
  accumulated across windows in an SBUF f32 accumulator.
- Layer 2 aggregates H (64-wide) first and applies W2 per output tile
  afterwards (A(HW) = (AH)W), keeping the L2 table at 64 real features.
"""

from dataclasses import dataclass

import numpy as np
import ml_dtypes

import concourse.bass as bass
import concourse.tile as tile
import concourse.bacc as bacc
from concourse import bass_utils, mybir
from concourse.masks import make_identity

F32 = mybir.dt.float32
BF16 = mybir.dt.bfloat16
I16 = mybir.dt.int16
P = 128
NWIN = 4   # src row windows (int16 index range)
NQ = 4     # group quarters (gather chunk granularity)


@dataclass(frozen=True)
class Cfg:
    n_nodes: int
    din: int
    dh: int
    dout: int
    C: int
    NG: int   # 128-dst groups per core (= node tiles)
    TBW: int  # 128-edge blocks per (group, window)

    @property
    def npc(self):
        return self.n_nodes // self.C

    @property
    def npcp(self):
        return self.NG * P

    @property
    def qsplits(self):
        base = self.NG // NQ
        rem = self.NG % NQ
        return tuple(base + (1 if q < rem else 0) for q in range(NQ))

    @property
    def qstarts(self):
        s, out = 0, []
        for q in self.qsplits:
            out.append(s)
            s += q
        return tuple(out)

    @property
    def winrows(self):  # table rows per window
        return self.npcp * self.C // NWIN

    @property
    def NB(self):  # total blocks per layer
        return self.NG * NWIN * self.TBW


def _balance_groups(degw_local, NG):
    """Assign npc nodes to NG groups of <=128, balancing per-window degree."""
    npc = len(degw_local)
    tot = degw_local.sum(axis=1)
    order = np.argsort(-tot, kind="stable")
    sums = np.zeros((NG, degw_local.shape[1]), dtype=np.int64)
    cnts = np.zeros(NG, dtype=np.int64)
    binof = np.zeros(npc, dtype=np.int64)
    for nidx in order:
        cand = sums + degw_local[nidx][None, :]
        score = cand.max(axis=1).astype(np.float64)
        score[cnts >= P] = np.inf
        b = int(np.argmin(score))
        binof[nidx] = b
        cnts[b] += 1
        sums[b] += degw_local[nidx]

    # swap refinement: push per-(group,window) counts under the next-lower
    # 128 multiple where possible
    target = ((int(sums.max()) - 1) // P) * P
    nodes_of = [np.where(binof == b)[0] for b in range(NG)]
    for _ in range(4000):
        mx = sums.max()
        if mx <= target:
            break
        g, w = np.unravel_index(int(np.argmax(sums)), sums.shape)
        best = None
        du = degw_local[nodes_of[g]]
        us = np.argsort(-du[:, w])[:24]
        for gp in np.argsort(sums[:, w])[:6]:
            gp = int(gp)
            if gp == g:
                continue
            dv = degw_local[nodes_of[gp]]
            for ui in us:
                d_u = du[ui]
                newg = sums[g] - d_u + dv  # [nv, W]
                newgp = sums[gp] + d_u - dv
                m = np.maximum(newg, newgp).max(axis=1)
                vi = int(np.argmin(m))
                if best is None or m[vi] < best[0]:
                    best = (m[vi], ui, vi, gp)
        if best is None or best[0] >= mx:
            break
        _, ui, vi, gp = best
        u = nodes_of[g][ui]
        v = nodes_of[gp][vi]
        sums[g] += degw_local[v] - degw_local[u]
        sums[gp] += degw_local[u] - degw_local[v]
        binof[u], binof[v] = gp, g
        nodes_of[g][ui] = v
        nodes_of[gp][vi] = u
    return binof


def build_schedule(edge_index: np.ndarray, n_nodes: int, C: int):
    # self-loops are handled analytically on-device; only real edges here
    src = np.asarray(edge_index[0], dtype=np.int64)
    dst = np.asarray(edge_index[1], dtype=np.int64)

    npc = n_nodes // C
    NG = -(-npc // P)
    npcp = NG * P

    # degree INCLUDING self-loop (for dinv), matching the reference
    deg = 1.0 + np.bincount(dst, minlength=n_nodes).astype(np.float32)
    owner = dst // npc

    # per-node per-window dst-degree: window of an edge = owner(src)//2
    ewin = (src // npc) // (C // NWIN)
    degw = np.zeros((n_nodes, NWIN), dtype=np.int64)
    np.add.at(degw, (dst, ewin), 1)

    pos_of_node = np.zeros((C, npc), dtype=np.int64)
    node_of_pos = np.full((C, npcp), -1, dtype=np.int64)
    for c in range(C):
        binof = _balance_groups(degw[c * npc : (c + 1) * npc], NG)
        order = np.argsort(binof, kind="stable")
        slot_in_bin = np.zeros(npc, dtype=np.int64)
        counts = np.bincount(binof, minlength=NG)
        starts = np.cumsum(counts) - counts
        pos_sorted = np.arange(npc) - starts[binof[order]]
        slot_in_bin[order] = pos_sorted
        pos = binof * P + slot_in_bin
        pos_of_node[c] = pos
        node_of_pos[c, pos] = np.arange(npc)

    winrows = npcp * C // NWIN
    growp = (np.arange(n_nodes) // npc) * npcp + pos_of_node[
        np.arange(n_nodes) // npc, np.arange(n_nodes) % npc
    ]
    s_row = growp[src]
    s_win = s_row // winrows
    s_idx = s_row % winrows

    d_pos = pos_of_node[owner, dst % npc]
    d_grp = d_pos // P
    d_rel = d_pos % P

    flat = ((owner * NG + d_grp) * NWIN + s_win).astype(np.int64)
    counts = np.bincount(flat, minlength=C * NG * NWIN)
    TBW = int(-(-counts.max() // P))
    cfg = Cfg(n_nodes=n_nodes, din=128, dh=64, dout=32, C=C, NG=NG, TBW=TBW)
    NB = cfg.NB
    qsplits, qstarts = cfg.qsplits, cfg.qstarts

    # group -> quarter, block-column offset of (g, w) run
    g_q = np.zeros(NG, dtype=np.int64)
    for q in range(NQ):
        g_q[qstarts[q] : qstarts[q] + qsplits[q]] = q
    # chunk ci = q*NWIN + w holds qsplits[q]*TBW blocks
    chunk_nblk = np.array([qsplits[ci // NWIN] * TBW for ci in range(NQ * NWIN)])
    chunk_b0 = np.cumsum(chunk_nblk) - chunk_nblk  # global block col offset
    # idx columns per chunk (wrapped/16)
    chunk_ncol = chunk_nblk * P // 16
    chunk_c0 = np.cumsum(chunk_ncol) - chunk_ncol
    GIDXCOLS = int(chunk_ncol.sum())

    order = np.lexsort((s_win, d_grp, owner))
    e_sorted = order
    starts = np.cumsum(counts) - counts
    rank = np.arange(len(src)) - starts[flat[order]]

    data = []
    for c in range(C):
        m = owner[e_sorted] == c
        es = e_sorted[m]
        rk = rank[m]
        g = d_grp[es]
        w = s_win[es]
        j = rk // P
        s = rk % P
        assert (j < TBW).all()
        q = g_q[g]
        ci = q * NWIN + w
        blk_in_chunk = (g - np.array(qstarts)[q]) * TBW + j
        i_in_chunk = blk_in_chunk * P + s
        col_b = chunk_b0[ci] + blk_in_chunk

        gidx = np.zeros(GIDXCOLS * 16, dtype=np.int16)
        gidx[(chunk_c0[ci] * 16 + i_in_chunk)] = s_idx[es].astype(np.int16)
        dst_rel = np.full((P, NB), -1.0, dtype=np.float32)
        dst_rel[s, col_b] = d_rel[es].astype(np.float32)

        # wrap16 per chunk
        gidx_sb = np.zeros((16, GIDXCOLS), dtype=np.int16)
        for ci2 in range(NQ * NWIN):
            c0, ncol = chunk_c0[ci2], chunk_ncol[ci2]
            a = gidx[c0 * 16 : (c0 + ncol) * 16].reshape(ncol, 16)
            gidx_sb[:, c0 : c0 + ncol] = a.T
        gidx_sb = np.tile(gidx_sb, (8, 1))

        deg_nm = np.ones(npcp, np.float32)
        valid = node_of_pos[c] >= 0
        deg_nm[valid] = deg[c * npc + node_of_pos[c][valid]]

        data.append(
            {
                "gidx": gidx_sb,
                "dst_rel": dst_rel.astype(ml_dtypes.bfloat16),
                "deg_nm": deg_nm.reshape(NG, P).T.copy(),  # [128, NG]
                "pos_of_node": pos_of_node[c],
            }
        )
    return cfg, data


def build_inputs(cfg: Cfg, x, W1, b1, W2, b2, sched):
    C, npc, npcp = cfg.C, cfg.npc, cfg.npcp
    x = np.asarray(x, dtype=np.float32)
    b1r = np.tile(np.asarray(b1, np.float32)[None, :], (P, 1))
    b2r = np.tile(np.asarray(b2, np.float32)[None, :], (P, 1))
    in_maps = []
    for c in range(C):
        xT = np.zeros((P, npcp), dtype=np.float32)
        pos = sched[c]["pos_of_node"]
        xT[:, pos] = x[c * npc : (c + 1) * npc].T
        in_maps.append(
            {
                "xT": xT,
                "W1in": np.asarray(W1, np.float32),
                "W2in": np.asarray(W2, np.float32),
                "b1in": b1r,
                "b2in": b2r,
                "gIdx": sched[c]["gidx"],
                "dstRel": sched[c]["dst_rel"],
                "degNM": sched[c]["deg_nm"],
            }
        )
    return in_maps


def build_nc(cfg: Cfg):
    C, TBW, DH, DOUT = cfg.C, cfg.TBW, cfg.dh, cfg.dout
    NG, NB = cfg.NG, cfg.NB
    npcp, winrows = cfg.npcp, cfg.winrows
    qsplits, qstarts = cfg.qsplits, cfg.qstarts
    chunk_nblk = [qsplits[ci // NWIN] * TBW for ci in range(NQ * NWIN)]
    chunk_b0 = np.cumsum(chunk_nblk) - np.array(chunk_nblk)
    chunk_ncol = [nb * P // 16 for nb in chunk_nblk]
    chunk_c0 = np.cumsum(chunk_ncol) - np.array(chunk_ncol)
    GIDXCOLS = int(sum(chunk_ncol))

    nc = bacc.Bacc("TRN2", target_bir_lowering=False, debug=False, num_devices=C)

    xT = nc.dram_tensor("xT", [P, npcp], F32, kind="ExternalInput").ap()
    W1in = nc.dram_tensor("W1in", [P, DH], F32, kind="ExternalInput").ap()
    W2in = nc.dram_tensor("W2in", [DH, DOUT], F32, kind="ExternalInput").ap()
    b1in = nc.dram_tensor("b1in", [P, DH], F32, kind="ExternalInput").ap()
    b2in = nc.dram_tensor("b2in", [P, DOUT], F32, kind="ExternalInput").ap()
    gIdx = nc.dram_tensor("gIdx", [P, GIDXCOLS], I16, kind="ExternalInput").ap()
    dstRel = nc.dram_tensor("dstRel", [P, NB], BF16, kind="ExternalInput").ap()
    degNM = nc.dram_tensor("degNM", [P, NG], F32, kind="ExternalInput").ap()
    z = nc.dram_tensor("z", [npcp, DOUT], F32, kind="ExternalOutput").ap()

    with tile.TileContext(nc) as tc:
        with (
            tc.tile_pool(name="const", bufs=1) as cpool,
            tc.tile_pool(name="work", bufs=1) as wpool,
            tc.tile_pool(name="psum", bufs=1, space="PSUM") as ppool,
            tc.tile_pool(name="dram", bufs=1, space="DRAM") as dpool,
        ):
            W1sb = cpool.tile([P, DH], F32)
            nc.sync.dma_start(W1sb[:], W1in[:])
            W2sb = cpool.tile([DH, DOUT], F32)
            nc.sync.dma_start(W2sb[:], W2in[:])
            b1sb = cpool.tile([P, DH], F32)
            nc.sync.dma_start(b1sb[:], b1in[:])
            b2sb = cpool.tile([P, DOUT], F32)
            nc.sync.dma_start(b2sb[:], b2in[:])
            identf = cpool.tile([P, P], F32)
            make_identity(nc, identf[:])
            iota_i = cpool.tile([P, P], mybir.dt.int32)
            nc.gpsimd.iota(iota_i[:], pattern=[[1, P]], base=0, channel_multiplier=0)
            iota_b = cpool.tile([P, P], BF16)
            nc.vector.tensor_copy(iota_b[:], iota_i[:])
            dst_sb = cpool.tile([P, NB], BF16)
            nc.sync.dma_start(dst_sb[:], dstRel[:])
            dinv = cpool.tile([P, NG], F32)
            nc.sync.dma_start(dinv[:], degNM[:])
            nc.scalar.activation(dinv[:], dinv[:], mybir.ActivationFunctionType.Sqrt)
            nc.vector.reciprocal(dinv[:], dinv[:])

            # own-row tables (self-loop terms), f32
            gown1 = cpool.tile([P, NG * DH], F32)
            gown2 = cpool.tile([P, NG * DH], F32)
            acc_sb = cpool.tile([P, NG * DH], F32)

            G1s = dpool.tile([npcp, P], BF16)
            G1f = dpool.tile([C * npcp, P], BF16)
            G2s = dpool.tile([npcp, P], BF16)
            G2f = dpool.tile([C * npcp, P], BF16)
            rg = [list(range(C))]

            # ---- layer 1 dense: G1 = dinv * (x @ W1), bf16, 128-wide ----
            for t in range(NG):
                ps = ppool.tile([P, DH], F32, tag="pdense", bufs=2, space="PSUM")
                xt_t = wpool.tile([P, P], F32, tag="xt", bufs=3)
                nc.sync.dma_start(xt_t[:], xT[:, t * P : (t + 1) * P])
                nc.tensor.matmul(ps[:], lhsT=xt_t[:], rhs=W1sb[:], start=True, stop=True)
                go = gown1[:, t * DH : (t + 1) * DH]
                nc.vector.tensor_scalar_mul(go, ps[:], dinv[:, t : t + 1])
                stage = wpool.tile([P, P], BF16, tag="g1stage", bufs=3)
                nc.vector.tensor_copy(stage[:, 0:DH], go)
                nc.vector.memset(stage[:, DH:P], 0.0)
                nc.sync.dma_start(G1s[t * P : (t + 1) * P, :], stage[:])

            nc.gpsimd.collective_compute(
                "AllGather", mybir.AluOpType.bypass, replica_groups=rg,
                ins=[G1s[:]], outs=[G1f[:]],
            )

            # ---- aggregation pass ----
            def agg_pass(Gf, gown, finish):
                for q in range(NQ):
                    qn, q0 = qsplits[q], qstarts[q]
                    for w in range(NWIN):
                        ci = q * NWIN + w
                        ncol = chunk_ncol[ci]
                        c0 = int(chunk_c0[ci])
                        b0 = int(chunk_b0[ci])
                        gi = wpool.tile([P, ncol], I16, tag="gi", bufs=2)
                        nc.sync.dma_start(gi[:], gIdx[:, c0 : c0 + ncol])
                        rows = wpool.tile([P, qn * TBW, P], BF16, tag="rows", bufs=2)
                        nc.gpsimd.dma_gather(
                            out_ap=rows[:],
                            in_ap=Gf[w * winrows : (w + 1) * winrows, :],
                            idxs_ap=gi[:],
                            num_idxs=qn * TBW * P,
                            num_idxs_reg=qn * TBW * P,
                            elem_size=P,
                            single_packet=False,
                        )
                        for gl in range(qn):
                            g = q0 + gl
                            ps = ppool.tile(
                                [P, DH], F32, tag="pacc", bufs=4, space="PSUM"
                            )
                            for j in range(TBW):
                                b = b0 + gl * TBW + j
                                S = wpool.tile([P, P], BF16, tag="S", bufs=6)
                                eng = nc.vector
                                eng.tensor_tensor(
                                    out=S[:],
                                    in0=dst_sb[:, b : b + 1].to_broadcast([P, P]),
                                    in1=iota_b[:],
                                    op=mybir.AluOpType.is_equal,
                                )
                                nc.tensor.matmul(
                                    ps[:],
                                    lhsT=S[:],
                                    rhs=rows[:, gl * TBW + j, 0:DH],
                                    start=(j == 0),
                                    stop=(j == TBW - 1),
                                )
                            asl = acc_sb[:, g * DH : (g + 1) * DH]
                            if w == 0:
                                # first window: acc = psum + own row (self loop)
                                nc.vector.tensor_tensor(
                                    out=asl, in0=ps[:],
                                    in1=gown[:, g * DH : (g + 1) * DH],
                                    op=mybir.AluOpType.add,
                                )
                            else:
                                nc.vector.tensor_tensor(
                                    out=asl, in0=asl, in1=ps[:],
                                    op=mybir.AluOpType.add,
                                )
                    for gl in range(qn):
                        g = q0 + gl
                        finish(g, acc_sb[:, g * DH : (g + 1) * DH])

            # ---- layer 1 finish -> G2 = dinv * relu(dinv*agg + b1) ----
            def finish1(t, acc):
                tmp = wpool.tile([P, DH], F32, tag="f1tmp", bufs=3)
                nc.vector.tensor_tensor(
                    out=tmp[:], in0=acc, in1=b1sb[:], op=mybir.AluOpType.add
                )
                nc.scalar.activation(
                    tmp[:], tmp[:], mybir.ActivationFunctionType.Relu,
                    scale=dinv[:, t : t + 1],
                )
                go = gown2[:, t * DH : (t + 1) * DH]
                nc.vector.tensor_scalar_mul(go, tmp[:], dinv[:, t : t + 1])
                stage = wpool.tile([P, P], BF16, tag="g2stage", bufs=3)
                nc.vector.tensor_copy(stage[:, 0:DH], go)
                nc.vector.memset(stage[:, DH:P], 0.0)
                nc.sync.dma_start(G2s[t * P : (t + 1) * P, :], stage[:])

            agg_pass(G1f, gown1, finish1)

            nc.gpsimd.collective_compute(
                "AllGather", mybir.AluOpType.bypass, replica_groups=rg,
                ins=[G2s[:]], outs=[G2f[:]],
            )

            # ---- layer 2 finish -> z = (dinv*agg) @ W2 + b2 ----
            def finish2(t, acc):
                s2 = wpool.tile([P, DH], F32, tag="f2s", bufs=2)
                nc.vector.tensor_scalar_mul(s2[:], acc, dinv[:, t : t + 1])
                pt = ppool.tile([DH, P], F32, tag="ptr", bufs=1, space="PSUM")
                nc.tensor.transpose(out=pt[:], in_=s2[:], identity=identf[:])
                hT = wpool.tile([DH, P], F32, tag="hT", bufs=2)
                nc.vector.tensor_copy(hT[:], pt[:])
                zp = ppool.tile([P, DOUT], F32, tag="zp", bufs=1, space="PSUM")
                nc.tensor.matmul(zp[:], lhsT=hT[:], rhs=W2sb[:], start=True, stop=True)
                zs = wpool.tile([P, DOUT], F32, tag="zs", bufs=3)
                nc.vector.tensor_tensor(
                    out=zs[:], in0=zp[:], in1=b2sb[:], op=mybir.AluOpType.add
                )
                nc.sync.dma_start(z[t * P : (t + 1) * P, :], zs[:])

            agg_pass(G2f, gown2, finish2)

    nc.compile()
    return nc


N_CORES = 8
_NC_CACHE = {}


def _cached_nc(cfg):
    if cfg not in _NC_CACHE:
        _NC_CACHE[cfg] = build_nc(cfg)
    return _NC_CACHE[cfg]


def kernel(x, W1, b1, W2, b2, edge_index):
    x = np.asarray(x)
    n = x.shape[0]
    cfg, sched = build_schedule(np.asarray(edge_index), n, N_CORES)
    in_maps = build_inputs(cfg, x, W1, b1, W2, b2, sched)
    nc = _cached_nc(cfg)
    res = bass_utils.run_bass_kernel_spmd(nc, in_maps, core_ids=list(range(N_CORES)))
    z = np.concatenate(
        [res.results[c]["z"][sched[c]["pos_of_node"]] for c in range(N_CORES)], axis=0
    )
    return z.astype(np.float32)



# revision 11
# speedup vs baseline: 1.9031x; 1.9031x over previous
"""Distributed 2-layer GCN (GCLEncoder) on 8 Trainium2 NeuronCores — Bass/Tile.

kernel(**inputs) takes the FULL inputs (x [100000,128] f32, W1 [128,64],
b1 [64], W2 [64,32], b2 [32], edge_index [2,1600000] i32) and returns the
FULL output z [100000, 32] f32.

v3 design (vs v2 baseline):
- Destination nodes sharded contiguously across 8 cores (12500 each, packed
  into 98 groups of <=128). Two-tier edge-count caps per (group, window)
  cell: groups 0..7 hold up to 640 edges/window (5 blocks), groups 8..97 up
  to 512 (4 blocks) -> 212,992 gather slots/layer vs 250,880 uniform.
- Per-layer node tables G (64 real bf16 features in 256B-strided rows,
  upper halves never written/read), exchanged via AllGather.
- Gathers: one dma_gather per (octant, window) chunk (32/layer); gpsimd
  descriptor generation is the kernel bottleneck (~8-9ns/row).
- Aggregation accumulates in PSUM across all 4 windows of an octant
  (no SBUF accumulator, no vector adds). Octants of ~12 groups keep
  PSUM under 16KB/partition.
- One-hot S built on DVE in one tensor_tensor per (group, window) cell
  ([128, nblk*128] is_equal vs iota), ~105ns/block.
- Self-loop injected via identity matmul from the SBUF-resident own-row
  stage; biases injected via rank-1 (sqrt(deg) (x) b) matmuls, so the
  finishes are pure scalar-engine activations (Relu/Copy with per-dst
  dinv scale) -- the vector engine never reads PSUM (avoids PE-PSUM port
  contention which made DVE ops 10-100x slower in v2).
- Layer 2 aggregates transposed (aggT [64h, 128d] PSUM; lhsT=rows,
  rhs=S), so the output projection needs no transpose:
  z = dinv * (aggT^T @ W2) + b2 via one matmul + scalar Copy.
"""

from dataclasses import dataclass

import numpy as np
import ml_dtypes

import concourse.bass as bass
import concourse.tile as tile
import concourse.bacc as bacc
from concourse import bass_utils, mybir
from concourse.masks import make_identity

F32 = mybir.dt.float32
BF16 = mybir.dt.bfloat16
I16 = mybir.dt.int16
P = 128
NWIN = 4
NOCT = 8
OVG = 8          # overflow groups (5-block cells); rest are 4-block
CAP_HI = 640
CAP_LO = 512


@dataclass(frozen=True)
class Cfg:
    n_nodes: int
    din: int
    dh: int
    dout: int
    C: int
    NG: int
    nblk: tuple  # per-cell block count, cell = g * NWIN + w

    @property
    def npc(self):
        return self.n_nodes // self.C

    @property
    def npcp(self):
        return self.NG * P

    @property
    def NB(self):
        return int(sum(self.nblk))

    @property
    def oct_groups(self):
        base = self.NG // NOCT
        rem = self.NG % NOCT
        sizes = [base + (1 if o < rem else 0) for o in range(NOCT)]
        out, s = [], 0
        for sz in sizes:
            out.append((s, sz))
            s += sz
        return tuple(out)

    @property
    def winrows(self):
        return self.npcp * self.C // NWIN

    def cell_nblk(self, g, w):
        return self.nblk[g * NWIN + w]

    @property
    def block_col(self):
        """column offset of each cell's blocks in dstRel, ordered
        (oct, w, g-in-oct, j)."""
        col = {}
        b = 0
        for o, (g0, gn) in enumerate(self.oct_groups):
            for w in range(NWIN):
                for g in range(g0, g0 + gn):
                    col[(g, w)] = b
                    b += self.cell_nblk(g, w)
        return col

    @property
    def chunk_info(self):
        """per (oct, w): (block col offset, nblk total, idx col offset)."""
        out = {}
        b = 0
        for o, (g0, gn) in enumerate(self.oct_groups):
            for w in range(NWIN):
                nb = sum(self.cell_nblk(g, w) for g in range(g0, g0 + gn))
                out[(o, w)] = (b, nb, b * 8)
                b += nb
        return out


def _balance_groups(degw_local, NG, caps):
    """Assign npc nodes to NG groups (<=128 nodes each) s.t. per-(g,w)
    edge counts stay under caps[g]. Greedy + repair."""
    npc, W = degw_local.shape
    order = np.argsort(-degw_local.sum(axis=1), kind="stable")
    sums = np.zeros((NG, W), dtype=np.int64)
    cnts = np.zeros(NG, dtype=np.int64)
    binof = np.full(npc, -1, dtype=np.int64)
    capv = caps[:, None]

    for nidx in order:
        d = degw_local[nidx][None, :]
        cand = sums + d
        over = np.maximum(cand - capv, 0).sum(axis=1)
        # prefer no violation, then lowest relative fill
        score = over * 1e6 + (cand / capv).max(axis=1)
        score[cnts >= P] = np.inf
        b = int(np.argmin(score))
        binof[nidx] = b
        cnts[b] += 1
        sums[b] += degw_local[nidx]

    nodes_of = [list(np.where(binof == b)[0]) for b in range(NG)]
    rng = np.random.default_rng(12345)
    for it in range(60000):
        viol = np.maximum(sums - capv, 0)
        tot_v = viol.sum()
        if tot_v == 0:
            break
        g, w = np.unravel_index(int(np.argmax(viol)), viol.shape)
        # try moving a node out of g into a group with room (and node space)
        du = degw_local[nodes_of[g]]
        u_order = np.argsort(-du[:, w])[:16]
        room = capv - sums  # [NG, W]
        done = False
        for ui in u_order:
            d_u = du[ui]
            fits = (room >= d_u[None, :]).all(axis=1) & (cnts < P)
            fits[g] = False
            if fits.any():
                cands = np.where(fits)[0]
                gp = int(cands[int(np.argmin((sums[cands] / capv[cands]).max(axis=1)))])
                u = nodes_of[g][ui]
                nodes_of[g].pop(ui)
                nodes_of[gp].append(u)
                binof[u] = gp
                sums[g] -= d_u
                sums[gp] += d_u
                cnts[g] -= 1
                cnts[gp] += 1
                done = True
                break
        if done:
            continue
        # swap: node u in g with node v in gp s.t. violations shrink
        best = None
        ui = int(u_order[0])
        d_u = degw_local[nodes_of[g][ui]]
        for gp in rng.permutation(NG)[:24]:
            gp = int(gp)
            if gp == g:
                continue
            dv = degw_local[nodes_of[gp]]
            ng = sums[g] - d_u[None, :] + dv
            ngp = sums[gp] + d_u[None, :] - dv
            v_new = (np.maximum(ng - capv[g], 0).sum(axis=1)
                     + np.maximum(ngp - capv[gp], 0).sum(axis=1))
            vi = int(np.argmin(v_new))
            base_v = viol[g].sum() + viol[gp].sum()
            if v_new[vi] < base_v and (best is None or v_new[vi] < best[0]):
                best = (v_new[vi], vi, gp)
        if best is None:
            continue
        _, vi, gp = best
        u = nodes_of[g][ui]
        v = nodes_of[gp][vi]
        sums[g] += degw_local[v] - degw_local[u]
        sums[gp] += degw_local[u] - degw_local[v]
        binof[u], binof[v] = gp, g
        nodes_of[g][ui] = v
        nodes_of[gp][vi] = u
    ok = (np.maximum(sums - capv, 0).sum() == 0)
    return binof, ok


def build_schedule(edge_index: np.ndarray, n_nodes: int, C: int):
    src = np.asarray(edge_index[0], dtype=np.int64)
    dst = np.asarray(edge_index[1], dtype=np.int64)
    npc = n_nodes // C
    NG = -(-npc // P)
    npcp = NG * P

    deg = 1.0 + np.bincount(dst, minlength=n_nodes).astype(np.float32)
    owner = dst // npc
    ewin = (src // npc) // (C // NWIN)
    degw = np.zeros((n_nodes, NWIN), dtype=np.int64)
    np.add.at(degw, (dst, ewin), 1)

    caps = np.full(NG, CAP_LO, dtype=np.int64)
    caps[:OVG] = CAP_HI

    pos_of_node = np.zeros((C, npc), dtype=np.int64)
    node_of_pos = np.full((C, npcp), -1, dtype=np.int64)
    cell_cnt = np.zeros((C, NG, NWIN), dtype=np.int64)
    for c in range(C):
        degw_c = degw[c * npc : (c + 1) * npc]
        binof, ok = _balance_groups(degw_c, NG, caps)
        order = np.argsort(binof, kind="stable")
        counts = np.bincount(binof, minlength=NG)
        starts = np.cumsum(counts) - counts
        slot = np.zeros(npc, dtype=np.int64)
        slot[order] = np.arange(npc) - starts[binof[order]]
        pos = binof * P + slot
        pos_of_node[c] = pos
        node_of_pos[c, pos] = np.arange(npc)
        for w in range(NWIN):
            np.add.at(cell_cnt[c, :, w], binof, degw_c[:, w])

    # per-cell blocks = max over cores (SPMD shares one NEFF)
    nblk = tuple(
        int(-(-cell_cnt[:, g, w].max() // P)) for g in range(NG) for w in range(NWIN)
    )
    cfg = Cfg(n_nodes=n_nodes, din=128, dh=64, dout=32, C=C, NG=NG, nblk=nblk)
    NB = cfg.NB
    block_col = cfg.block_col
    chunk_info = cfg.chunk_info
    winrows = cfg.winrows

    growp = (np.arange(n_nodes) // npc) * npcp + pos_of_node[
        np.arange(n_nodes) // npc, np.arange(n_nodes) % npc
    ]
    s_row = growp[src]
    s_win = ewin  # window = owner(src) pair, independent of balance
    s_idx = s_row % winrows

    d_pos = pos_of_node[owner, dst % npc]
    d_grp = d_pos // P
    d_rel = d_pos % P

    # rank within (owner, d_grp, s_win)
    flat = ((owner * NG + d_grp) * NWIN + s_win).astype(np.int64)
    order = np.lexsort((s_win, d_grp, owner))
    counts = np.bincount(flat, minlength=C * NG * NWIN)
    starts = np.cumsum(counts) - counts
    rank = np.arange(len(src)) - starts[flat[order]]

    cell_nblk_arr = np.array(cfg.nblk, dtype=np.int64).reshape(NG, NWIN)
    col0 = np.zeros((NG, NWIN), dtype=np.int64)
    for (g, w), b in block_col.items():
        col0[g, w] = b
    GIDXCOLS = NB * 8

    data = []
    for c in range(C):
        m = owner[order] == c
        es = order[m]
        rk = rank[m]
        g = d_grp[es]
        w = s_win[es]
        assert (rk < cell_nblk_arr[g, w] * P).all(), "cell overflow"
        j = rk // P
        s = rk % P
        col_b = col0[g, w] + j

        gidx = np.zeros(NB * P, dtype=np.int16)
        gidx[col_b * P + s] = s_idx[es].astype(np.int16)
        dst_rel = np.full((P, NB), -1.0, dtype=np.float32)
        dst_rel[s, col_b] = d_rel[es].astype(np.float32)

        # wrap16 per chunk
        gidx_sb = np.zeros((16, GIDXCOLS), dtype=np.int16)
        for (o, wc), (b0, nb, c0) in chunk_info.items():
            a = gidx[b0 * P : (b0 + nb) * P].reshape(nb * 8, 16)
            gidx_sb[:, c0 : c0 + nb * 8] = a.T
        gidx_sb = np.tile(gidx_sb, (8, 1))

        deg_nm = np.ones(npcp, np.float32)
        valid = node_of_pos[c] >= 0
        deg_nm[valid] = deg[c * npc + node_of_pos[c][valid]]

        data.append(
            {
                "gidx": gidx_sb,
                "dst_rel": dst_rel.astype(ml_dtypes.bfloat16),
                "deg_nm": deg_nm,
                "pos_of_node": pos_of_node[c],
            }
        )
    return cfg, data


def build_inputs(cfg: Cfg, x, W1, b1, W2, b2, sched):
    C, npc, npcp = cfg.C, cfg.npc, cfg.npcp
    x = np.asarray(x, dtype=np.float32)
    in_maps = []
    for c in range(C):
        xT = np.zeros((P, npcp), dtype=np.float32)
        pos = sched[c]["pos_of_node"]
        xT[:, pos] = x[c * npc : (c + 1) * npc].T
        deg = sched[c]["deg_nm"]
        dinv = (1.0 / np.sqrt(deg)).astype(np.float32)
        in_maps.append(
            {
                "xT": xT,
                "W1in": np.asarray(W1, np.float32),
                "W2in": np.asarray(W2, np.float32),
                "b1in": np.asarray(b1, np.float32)[None, :],
                "b2in": np.asarray(b2, np.float32)[None, :],
                "gIdx": sched[c]["gidx"],
                "dstRel": sched[c]["dst_rel"],
                "dinvNM": dinv.reshape(cfg.NG, P).T.copy(),   # [128, NG]
                "sqdQ": np.sqrt(deg).astype(np.float32)[None, :],  # [1, npcp]
            }
        )
    return in_maps


def build_nc(cfg: Cfg):
    C, DH, DOUT = cfg.C, cfg.dh, cfg.dout
    NG, NB, npcp, winrows = cfg.NG, cfg.NB, cfg.npcp, cfg.winrows
    oct_groups = cfg.oct_groups
    block_col = cfg.block_col
    chunk_info = cfg.chunk_info
    GIDXCOLS = NB * 8
    max_chunk_nb = max(nb for (_, nb, _) in chunk_info.values())
    max_cell_nb = max(cfg.nblk)

    nc = bacc.Bacc("TRN2", target_bir_lowering=False, debug=False, num_devices=C)

    xT = nc.dram_tensor("xT", [P, npcp], F32, kind="ExternalInput").ap()
    W1in = nc.dram_tensor("W1in", [P, DH], F32, kind="ExternalInput").ap()
    W2in = nc.dram_tensor("W2in", [DH, DOUT], F32, kind="ExternalInput").ap()
    b1in = nc.dram_tensor("b1in", [1, DH], F32, kind="ExternalInput").ap()
    b2in = nc.dram_tensor("b2in", [1, DOUT], F32, kind="ExternalInput").ap()
    gIdx = nc.dram_tensor("gIdx", [P, GIDXCOLS], I16, kind="ExternalInput").ap()
    dstRel = nc.dram_tensor("dstRel", [P, NB], BF16, kind="ExternalInput").ap()
    dinvNM = nc.dram_tensor("dinvNM", [P, NG], F32, kind="ExternalInput").ap()
    sqdQ = nc.dram_tensor("sqdQ", [1, npcp], F32, kind="ExternalInput").ap()
    z = nc.dram_tensor("z", [npcp, DOUT], F32, kind="ExternalOutput").ap()

    with tile.TileContext(nc) as tc:
        with (
            tc.tile_pool(name="const", bufs=1) as cpool,
            tc.tile_pool(name="work", bufs=1) as wpool,
            tc.tile_pool(name="psum", bufs=1, space="PSUM") as ppool,
            tc.tile_pool(name="dram", bufs=1, space="DRAM") as dpool,
        ):
            W1sb = cpool.tile([P, DH], F32)
            nc.sync.dma_start(W1sb[:], W1in[:])
            W2sb = cpool.tile([DH, DOUT], F32)
            nc.sync.dma_start(W2sb[:], W2in[:])
            b1sb = cpool.tile([1, DH], F32)
            nc.sync.dma_start(b1sb[:], b1in[:])
            b2sb = cpool.tile([1, DOUT], F32)
            nc.sync.dma_start(b2sb[:], b2in[:])
            dinv = cpool.tile([P, NG], F32)
            nc.sync.dma_start(dinv[:], dinvNM[:])
            sqd = cpool.tile([1, npcp], F32)
            nc.sync.dma_start(sqd[:], sqdQ[:])
            dst_sb = cpool.tile([P, NB], BF16)
            nc.sync.dma_start(dst_sb[:], dstRel[:])
            gidx_sb = cpool.tile([P, GIDXCOLS], I16)
            nc.sync.dma_start(gidx_sb[:], gIdx[:])

            zrow = cpool.tile([1, 512], F32)
            nc.vector.memset(zrow[:], 0.0)
            identb = cpool.tile([P, P], BF16)
            make_identity(nc, identb[:])
            iota_i = cpool.tile([P, P], mybir.dt.int32)
            nc.gpsimd.iota(iota_i[:], pattern=[[1, P]], base=0, channel_multiplier=0)
            iota_b = cpool.tile([P, P], BF16)
            nc.vector.tensor_copy(iota_b[:], iota_i[:])

            stage1 = cpool.tile([P, NG * DH], BF16)   # G1 own rows (table vals)
            stage2 = cpool.tile([P, NG * DH], BF16)   # G2 own rows

            G1s = dpool.tile([npcp, P], BF16)
            G1f = dpool.tile([C * npcp, P], BF16)
            G2s = dpool.tile([npcp, P], BF16)
            G2f = dpool.tile([C * npcp, P], BF16)
            rg = [list(range(C))]

            # ---- layer 1 dense: stage1 = dinv * (x @ W1) (bf16), to G1s ----
            for g in range(NG):
                ps = ppool.tile([P, DH], F32, tag="pdense", bufs=1, space="PSUM")
                xt_t = wpool.tile([P, P], F32, tag="xt", bufs=3)
                nc.sync.dma_start(xt_t[:], xT[:, g * P : (g + 1) * P])
                nc.tensor.matmul(ps[:], lhsT=xt_t[:], rhs=W1sb[:], start=True, stop=True)
                sl = stage1[:, g * DH : (g + 1) * DH]
                nc.scalar.activation(
                    sl, ps[:], mybir.ActivationFunctionType.Copy,
                    scale=dinv[:, g : g + 1],
                )
                nc.sync.dma_start(G1s[g * P : (g + 1) * P, 0:DH], sl)

            nc.gpsimd.collective_compute(
                "AllGather", mybir.AluOpType.bypass, replica_groups=rg,
                ins=[G1s[:]], outs=[G1f[:]],
            )

            # last nonempty (w, j) per group, for the PSUM stop flag
            last_wj = {}
            for g in range(NG):
                last_wj[g] = None
                for w in range(NWIN):
                    if cfg.cell_nblk(g, w) > 0:
                        last_wj[g] = (w, cfg.cell_nblk(g, w) - 1)

            def run_layer(Gf, layer):
                for o, (g0, gn) in enumerate(oct_groups):
                    # pack group accumulators into bank-sized PSUM tiles
                    psg_of = {}
                    if layer == 1:
                        nbank = -(-gn // 8)
                        banks = [
                            ppool.tile(
                                [P, 512], F32, tag=f"ps1b_{k}", bufs=1,
                                space="PSUM", name=f"psg1_{o}_{k}",
                            )
                            for k in range(nbank)
                        ]
                        for g in range(g0, g0 + gn):
                            i = g - g0
                            psg_of[g] = banks[i // 8][:, (i % 8) * DH : (i % 8 + 1) * DH]
                        for k in range(nbank):
                            # start=True resets the whole PSUM bank on HW, so
                            # zero each bank once and accumulate into slices
                            nc.tensor.matmul(
                                banks[k][:], lhsT=zrow[:, 0:P], rhs=zrow[:],
                                start=True, stop=False, skip_group_check=True,
                            )
                    else:
                        nbank = -(-gn // 4)
                        banks = [
                            ppool.tile(
                                [DH, 512], F32, tag=f"ps2b_{k}", bufs=1,
                                space="PSUM", name=f"psg2_{o}_{k}",
                            )
                            for k in range(nbank)
                        ]
                        for g in range(g0, g0 + gn):
                            i = g - g0
                            psg_of[g] = banks[i // 4][:, (i % 4) * P : (i % 4 + 1) * P]
                        for k in range(nbank):
                            nc.tensor.matmul(
                                banks[k][:], lhsT=zrow[:, 0:DH], rhs=zrow[:],
                                start=True, stop=False, skip_group_check=True,
                            )
                    for w in range(NWIN):
                        b0, nb, c0 = chunk_info[(o, w)]
                        if nb == 0:
                            continue
                        rows = wpool.tile(
                            [P, max_chunk_nb, P], BF16, tag="rows", bufs=2
                        )
                        nc.gpsimd.dma_gather(
                            out_ap=rows[:, 0:nb, :],
                            in_ap=Gf[w * winrows : (w + 1) * winrows, :],
                            idxs_ap=gidx_sb[:, c0 : c0 + nb * 8],
                            num_idxs=nb * P,
                            num_idxs_reg=nb * P,
                            elem_size=P,
                            single_packet=False,
                        )
                        jj = 0
                        for g in range(g0, g0 + gn):
                            cnb = cfg.cell_nblk(g, w)
                            bcol = block_col[(g, w)]
                            psg = psg_of[g]
                            first_w = (w == 0) or all(
                                cfg.cell_nblk(g, w2) == 0 for w2 in range(w)
                            )
                            if first_w:
                                # self-loop + bias injection opens the group
                                only = last_wj[g] is None
                                if layer == 1:
                                    nc.tensor.matmul(
                                        psg, lhsT=identb[:],
                                        rhs=stage1[:, g * DH : (g + 1) * DH],
                                        start=False, stop=False,
                                        skip_group_check=True,
                                    )
                                    nc.tensor.matmul(
                                        psg,
                                        lhsT=sqd[:, g * P : (g + 1) * P],
                                        rhs=b1sb[:],
                                        start=False, stop=only,
                                        skip_group_check=True,
                                    )
                                else:
                                    nc.tensor.matmul(
                                        psg,
                                        lhsT=stage2[:, g * DH : (g + 1) * DH],
                                        rhs=identb[:],
                                        start=False, stop=only,
                                        skip_group_check=True,
                                    )
                            if cnb == 0:
                                continue
                            S = wpool.tile(
                                [P, max_cell_nb, P], BF16, tag="S", bufs=4
                            )
                            nc.vector.tensor_tensor(
                                out=S[:, 0:cnb, :],
                                in0=dst_sb[:, bcol : bcol + cnb]
                                .unsqueeze(2).to_broadcast([P, cnb, P]),
                                in1=iota_b[:].unsqueeze(1).to_broadcast([P, cnb, P]),
                                op=mybir.AluOpType.is_equal,
                            )
                            for j in range(cnb):
                                last = last_wj[g] == (w, j)
                                if layer == 1:
                                    nc.tensor.matmul(
                                        psg, lhsT=S[:, j, :],
                                        rhs=rows[:, jj + j, 0:DH],
                                        start=False, stop=last,
                                        skip_group_check=True,
                                    )
                                else:
                                    nc.tensor.matmul(
                                        psg, lhsT=rows[:, jj + j, 0:DH],
                                        rhs=S[:, j, :],
                                        start=False, stop=last,
                                        skip_group_check=True,
                                    )
                            jj += cnb
                    # finish the octant's groups
                    for g in range(g0, g0 + gn):
                        psg = psg_of[g]
                        if layer == 1:
                            tmp = wpool.tile([P, DH], F32, tag="f1tmp", bufs=3)
                            nc.scalar.activation(
                                tmp[:], psg, mybir.ActivationFunctionType.Relu,
                                scale=dinv[:, g : g + 1],
                            )
                            sl = stage2[:, g * DH : (g + 1) * DH]
                            nc.scalar.activation(
                                sl, tmp[:], mybir.ActivationFunctionType.Copy,
                                scale=dinv[:, g : g + 1],
                            )
                            nc.sync.dma_start(G2s[g * P : (g + 1) * P, 0:DH], sl)
                        else:
                            aT = wpool.tile([DH, P], F32, tag="aT", bufs=3)
                            nc.scalar.activation(
                                aT[:], psg, mybir.ActivationFunctionType.Copy,
                            )
                            zp = ppool.tile(
                                [P, DOUT], F32, tag="zp", bufs=1, space="PSUM"
                            )
                            nc.tensor.matmul(
                                zp[:], lhsT=aT[:], rhs=W2sb[:],
                                start=True, stop=False,
                            )
                            nc.tensor.matmul(
                                zp[:], lhsT=sqd[:, g * P : (g + 1) * P],
                                rhs=b2sb[:], start=False, stop=True,
                            )
                            zs = wpool.tile([P, DOUT], F32, tag="zs", bufs=3)
                            nc.scalar.activation(
                                zs[:], zp[:], mybir.ActivationFunctionType.Copy,
                                scale=dinv[:, g : g + 1],
                            )
                            nc.sync.dma_start(z[g * P : (g + 1) * P, :], zs[:])

            run_layer(G1f, 1)

            nc.gpsimd.collective_compute(
                "AllGather", mybir.AluOpType.bypass, replica_groups=rg,
                ins=[G2s[:]], outs=[G2f[:]],
            )

            run_layer(G2f, 2)

    nc.compile()
    return nc


N_CORES = 8
_NC_CACHE = {}


def _cached_nc(cfg):
    if cfg not in _NC_CACHE:
        _NC_CACHE[cfg] = build_nc(cfg)
    return _NC_CACHE[cfg]


def kernel(x, W1, b1, W2, b2, edge_index):
    x = np.asarray(x)
    n = x.shape[0]
    cfg, sched = build_schedule(np.asarray(edge_index), n, N_CORES)
    in_maps = build_inputs(cfg, x, W1, b1, W2, b2, sched)
    nc = _cached_nc(cfg)
    res = bass_utils.run_bass_kernel_spmd(nc, in_maps, core_ids=list(range(N_CORES)))
    z = np.concatenate(
        [res.results[c]["z"][sched[c]["pos_of_node"]] for c in range(N_CORES)], axis=0
    )
    return z.astype(np.float32)


# revision 12
# speedup vs baseline: 1.9879x; 1.0446x over previous
"""Distributed 2-layer GCN (GCLEncoder) on 8 Trainium2 NeuronCores — Bass/Tile.

kernel(**inputs) takes the FULL inputs (x [100000,128] f32, W1 [128,64],
b1 [64], W2 [64,32], b2 [32], edge_index [2,1600000] i32) and returns the
FULL output z [100000, 32] f32.

v3 design (vs v2 baseline):
- Destination nodes sharded contiguously across 8 cores (12500 each, packed
  into 98 groups of <=128). Two-tier edge-count caps per (group, window)
  cell: groups 0..7 hold up to 640 edges/window (5 blocks), groups 8..97 up
  to 512 (4 blocks) -> 212,992 gather slots/layer vs 250,880 uniform.
- Per-layer node tables G (64 real bf16 features in 256B-strided rows,
  upper halves never written/read), exchanged via AllGather.
- Gathers: one dma_gather per (octant, window) chunk (32/layer); gpsimd
  descriptor generation is the kernel bottleneck (~8-9ns/row).
- Aggregation accumulates in PSUM across all 4 windows of an octant
  (no SBUF accumulator, no vector adds). Octants of ~12 groups keep
  PSUM under 16KB/partition.
- One-hot S built on DVE in one tensor_tensor per (group, window) cell
  ([128, nblk*128] is_equal vs iota), ~105ns/block.
- Self-loop injected via identity matmul from the SBUF-resident own-row
  stage; biases injected via rank-1 (sqrt(deg) (x) b) matmuls, so the
  finishes are pure scalar-engine activations (Relu/Copy with per-dst
  dinv scale) -- the vector engine never reads PSUM (avoids PE-PSUM port
  contention which made DVE ops 10-100x slower in v2).
- Layer 2 aggregates transposed (aggT [64h, 128d] PSUM; lhsT=rows,
  rhs=S), so the output projection needs no transpose:
  z = dinv * (aggT^T @ W2) + b2 via one matmul + scalar Copy.
"""

from dataclasses import dataclass

import numpy as np
import ml_dtypes

import concourse.bass as bass
import concourse.tile as tile
import concourse.bacc as bacc
from concourse import bass_utils, mybir
from concourse.masks import make_identity

F32 = mybir.dt.float32
BF16 = mybir.dt.bfloat16
I16 = mybir.dt.int16
P = 128
NWIN = 4
NOCT = 8
OVG = 4          # overflow groups (5-block cells); rest are 4-block
CAP_HI = 640
CAP_LO = 512


@dataclass(frozen=True)
class Cfg:
    n_nodes: int
    din: int
    dh: int
    dout: int
    C: int
    NG: int
    nblk: tuple  # per-cell block count, cell = g * NWIN + w

    @property
    def npc(self):
        return self.n_nodes // self.C

    @property
    def npcp(self):
        return self.NG * P

    @property
    def NB(self):
        return int(sum(self.nblk))

    @property
    def oct_groups(self):
        base = self.NG // NOCT
        rem = self.NG % NOCT
        sizes = [base + (1 if o < rem else 0) for o in range(NOCT)]
        out, s = [], 0
        for sz in sizes:
            out.append((s, sz))
            s += sz
        return tuple(out)

    @property
    def winrows(self):
        return self.npcp * self.C // NWIN

    def cell_nblk(self, g, w):
        return self.nblk[g * NWIN + w]

    @property
    def block_col(self):
        """column offset of each cell's blocks in dstRel, ordered
        (oct, w, g-in-oct, j)."""
        col = {}
        b = 0
        for o, (g0, gn) in enumerate(self.oct_groups):
            for w in range(NWIN):
                for g in range(g0, g0 + gn):
                    col[(g, w)] = b
                    b += self.cell_nblk(g, w)
        return col

    @property
    def chunk_info(self):
        """per (oct, w): (block col offset, nblk total, idx col offset)."""
        out = {}
        b = 0
        for o, (g0, gn) in enumerate(self.oct_groups):
            for w in range(NWIN):
                nb = sum(self.cell_nblk(g, w) for g in range(g0, g0 + gn))
                out[(o, w)] = (b, nb, b * 8)
                b += nb
        return out


def _balance_groups(degw_local, NG, caps):
    """Assign npc nodes to NG groups (<=128 nodes each) s.t. per-(g,w)
    edge counts stay under caps[g]. Greedy + repair."""
    npc, W = degw_local.shape
    order = np.argsort(-degw_local.sum(axis=1), kind="stable")
    sums = np.zeros((NG, W), dtype=np.int64)
    cnts = np.zeros(NG, dtype=np.int64)
    binof = np.full(npc, -1, dtype=np.int64)
    capv = caps[:, None]

    for nidx in order:
        d = degw_local[nidx][None, :]
        cand = sums + d
        over = np.maximum(cand - capv, 0).sum(axis=1)
        # prefer no violation, then lowest relative fill
        score = over * 1e6 + (cand / capv).max(axis=1)
        score[cnts >= P] = np.inf
        b = int(np.argmin(score))
        binof[nidx] = b
        cnts[b] += 1
        sums[b] += degw_local[nidx]

    nodes_of = [list(np.where(binof == b)[0]) for b in range(NG)]
    rng = np.random.default_rng(12345)
    for it in range(60000):
        viol = np.maximum(sums - capv, 0)
        tot_v = viol.sum()
        if tot_v == 0:
            break
        g, w = np.unravel_index(int(np.argmax(viol)), viol.shape)
        # try moving a node out of g into a group with room (and node space)
        du = degw_local[nodes_of[g]]
        u_order = np.argsort(-du[:, w])[:16]
        room = capv - sums  # [NG, W]
        done = False
        for ui in u_order:
            d_u = du[ui]
            fits = (room >= d_u[None, :]).all(axis=1) & (cnts < P)
            fits[g] = False
            if fits.any():
                cands = np.where(fits)[0]
                gp = int(cands[int(np.argmin((sums[cands] / capv[cands]).max(axis=1)))])
                u = nodes_of[g][ui]
                nodes_of[g].pop(ui)
                nodes_of[gp].append(u)
                binof[u] = gp
                sums[g] -= d_u
                sums[gp] += d_u
                cnts[g] -= 1
                cnts[gp] += 1
                done = True
                break
        if done:
            continue
        # swap: node u in g with node v in gp s.t. violations shrink
        best = None
        ui = int(u_order[0])
        d_u = degw_local[nodes_of[g][ui]]
        for gp in rng.permutation(NG)[:24]:
            gp = int(gp)
            if gp == g:
                continue
            dv = degw_local[nodes_of[gp]]
            ng = sums[g] - d_u[None, :] + dv
            ngp = sums[gp] + d_u[None, :] - dv
            v_new = (np.maximum(ng - capv[g], 0).sum(axis=1)
                     + np.maximum(ngp - capv[gp], 0).sum(axis=1))
            vi = int(np.argmin(v_new))
            base_v = viol[g].sum() + viol[gp].sum()
            if v_new[vi] < base_v and (best is None or v_new[vi] < best[0]):
                best = (v_new[vi], vi, gp)
        if best is None:
            continue
        _, vi, gp = best
        u = nodes_of[g][ui]
        v = nodes_of[gp][vi]
        sums[g] += degw_local[v] - degw_local[u]
        sums[gp] += degw_local[u] - degw_local[v]
        binof[u], binof[v] = gp, g
        nodes_of[g][ui] = v
        nodes_of[gp][vi] = u
    ok = (np.maximum(sums - capv, 0).sum() == 0)
    return binof, ok


def build_schedule(edge_index: np.ndarray, n_nodes: int, C: int):
    src = np.asarray(edge_index[0], dtype=np.int64)
    dst = np.asarray(edge_index[1], dtype=np.int64)
    npc = n_nodes // C
    NG = -(-npc // P)
    npcp = NG * P

    deg = 1.0 + np.bincount(dst, minlength=n_nodes).astype(np.float32)
    owner = dst // npc
    ewin = (src // npc) // (C // NWIN)
    degw = np.zeros((n_nodes, NWIN), dtype=np.int64)
    np.add.at(degw, (dst, ewin), 1)

    caps = np.full(NG, CAP_LO, dtype=np.int64)
    caps[:OVG] = CAP_HI

    pos_of_node = np.zeros((C, npc), dtype=np.int64)
    node_of_pos = np.full((C, npcp), -1, dtype=np.int64)
    cell_cnt = np.zeros((C, NG, NWIN), dtype=np.int64)
    for c in range(C):
        degw_c = degw[c * npc : (c + 1) * npc]
        binof, ok = _balance_groups(degw_c, NG, caps)
        order = np.argsort(binof, kind="stable")
        counts = np.bincount(binof, minlength=NG)
        starts = np.cumsum(counts) - counts
        slot = np.zeros(npc, dtype=np.int64)
        slot[order] = np.arange(npc) - starts[binof[order]]
        pos = binof * P + slot
        pos_of_node[c] = pos
        node_of_pos[c, pos] = np.arange(npc)
        for w in range(NWIN):
            np.add.at(cell_cnt[c, :, w], binof, degw_c[:, w])

    # per-cell blocks = max over cores (SPMD shares one NEFF)
    nblk = tuple(
        int(-(-cell_cnt[:, g, w].max() // P)) for g in range(NG) for w in range(NWIN)
    )
    cfg = Cfg(n_nodes=n_nodes, din=128, dh=64, dout=32, C=C, NG=NG, nblk=nblk)
    NB = cfg.NB
    block_col = cfg.block_col
    chunk_info = cfg.chunk_info
    winrows = cfg.winrows

    growp = (np.arange(n_nodes) // npc) * npcp + pos_of_node[
        np.arange(n_nodes) // npc, np.arange(n_nodes) % npc
    ]
    s_row = growp[src]
    s_win = ewin  # window = owner(src) pair, independent of balance
    s_idx = s_row % winrows

    d_pos = pos_of_node[owner, dst % npc]
    d_grp = d_pos // P
    d_rel = d_pos % P

    # rank within (owner, d_grp, s_win)
    flat = ((owner * NG + d_grp) * NWIN + s_win).astype(np.int64)
    order = np.lexsort((s_win, d_grp, owner))
    counts = np.bincount(flat, minlength=C * NG * NWIN)
    starts = np.cumsum(counts) - counts
    rank = np.arange(len(src)) - starts[flat[order]]

    cell_nblk_arr = np.array(cfg.nblk, dtype=np.int64).reshape(NG, NWIN)
    col0 = np.zeros((NG, NWIN), dtype=np.int64)
    for (g, w), b in block_col.items():
        col0[g, w] = b
    GIDXCOLS = NB * 8

    data = []
    for c in range(C):
        m = owner[order] == c
        es = order[m]
        rk = rank[m]
        g = d_grp[es]
        w = s_win[es]
        assert (rk < cell_nblk_arr[g, w] * P).all(), "cell overflow"
        j = rk // P
        s = rk % P
        col_b = col0[g, w] + j

        gidx = np.zeros(NB * P, dtype=np.int16)
        gidx[col_b * P + s] = s_idx[es].astype(np.int16)
        dst_rel = np.full((P, NB), -1.0, dtype=np.float32)
        dst_rel[s, col_b] = d_rel[es].astype(np.float32)

        # wrap16 per chunk
        gidx_sb = np.zeros((16, GIDXCOLS), dtype=np.int16)
        for (o, wc), (b0, nb, c0) in chunk_info.items():
            a = gidx[b0 * P : (b0 + nb) * P].reshape(nb * 8, 16)
            gidx_sb[:, c0 : c0 + nb * 8] = a.T
        gidx_sb = np.tile(gidx_sb, (8, 1))

        deg_nm = np.ones(npcp, np.float32)
        valid = node_of_pos[c] >= 0
        deg_nm[valid] = deg[c * npc + node_of_pos[c][valid]]

        data.append(
            {
                "gidx": gidx_sb,
                "dst_rel": dst_rel.astype(ml_dtypes.bfloat16),
                "deg_nm": deg_nm,
                "pos_of_node": pos_of_node[c],
            }
        )
    return cfg, data


def build_inputs(cfg: Cfg, x, W1, b1, W2, b2, sched):
    C, npc, npcp = cfg.C, cfg.npc, cfg.npcp
    x = np.asarray(x, dtype=np.float32)
    in_maps = []
    for c in range(C):
        xT = np.zeros((P, npcp), dtype=np.float32)
        pos = sched[c]["pos_of_node"]
        xT[:, pos] = x[c * npc : (c + 1) * npc].T
        deg = sched[c]["deg_nm"]
        dinv = (1.0 / np.sqrt(deg)).astype(np.float32)
        in_maps.append(
            {
                "xT": xT,
                "W1in": np.asarray(W1, np.float32),
                "W2in": np.asarray(W2, np.float32),
                "b1in": np.asarray(b1, np.float32)[None, :],
                "b2in": np.asarray(b2, np.float32)[None, :],
                "gIdx": sched[c]["gidx"],
                "dstRel": sched[c]["dst_rel"],
                "dinvNM": dinv.reshape(cfg.NG, P).T.copy(),   # [128, NG]
                "sqdQ": np.sqrt(deg).astype(np.float32)[None, :],  # [1, npcp]
            }
        )
    return in_maps


def build_nc(cfg: Cfg):
    C, DH, DOUT = cfg.C, cfg.dh, cfg.dout
    NG, NB, npcp, winrows = cfg.NG, cfg.NB, cfg.npcp, cfg.winrows
    oct_groups = cfg.oct_groups
    block_col = cfg.block_col
    chunk_info = cfg.chunk_info
    GIDXCOLS = NB * 8
    max_chunk_nb = max(nb for (_, nb, _) in chunk_info.values())
    max_cell_nb = max(cfg.nblk)

    nc = bacc.Bacc("TRN2", target_bir_lowering=False, debug=False, num_devices=C)

    xT = nc.dram_tensor("xT", [P, npcp], F32, kind="ExternalInput").ap()
    W1in = nc.dram_tensor("W1in", [P, DH], F32, kind="ExternalInput").ap()
    W2in = nc.dram_tensor("W2in", [DH, DOUT], F32, kind="ExternalInput").ap()
    b1in = nc.dram_tensor("b1in", [1, DH], F32, kind="ExternalInput").ap()
    b2in = nc.dram_tensor("b2in", [1, DOUT], F32, kind="ExternalInput").ap()
    gIdx = nc.dram_tensor("gIdx", [P, GIDXCOLS], I16, kind="ExternalInput").ap()
    dstRel = nc.dram_tensor("dstRel", [P, NB], BF16, kind="ExternalInput").ap()
    dinvNM = nc.dram_tensor("dinvNM", [P, NG], F32, kind="ExternalInput").ap()
    sqdQ = nc.dram_tensor("sqdQ", [1, npcp], F32, kind="ExternalInput").ap()
    z = nc.dram_tensor("z", [npcp, DOUT], F32, kind="ExternalOutput").ap()

    with tile.TileContext(nc) as tc:
        with (
            tc.tile_pool(name="const", bufs=1) as cpool,
            tc.tile_pool(name="work", bufs=1) as wpool,
            tc.tile_pool(name="psum", bufs=1, space="PSUM") as ppool,
            tc.tile_pool(name="dram", bufs=1, space="DRAM") as dpool,
        ):
            W1sb = cpool.tile([P, DH], F32)
            nc.sync.dma_start(W1sb[:], W1in[:])
            W2sb = cpool.tile([DH, DOUT], F32)
            nc.sync.dma_start(W2sb[:], W2in[:])
            b1sb = cpool.tile([1, DH], F32)
            nc.sync.dma_start(b1sb[:], b1in[:])
            b2sb = cpool.tile([1, DOUT], F32)
            nc.sync.dma_start(b2sb[:], b2in[:])
            dinv = cpool.tile([P, NG], F32)
            nc.sync.dma_start(dinv[:], dinvNM[:])
            sqd = cpool.tile([1, npcp], F32)
            nc.sync.dma_start(sqd[:], sqdQ[:])
            dst_sb = cpool.tile([P, NB], BF16)
            nc.sync.dma_start(dst_sb[:], dstRel[:])
            gidx_sb = cpool.tile([P, GIDXCOLS], I16)
            nc.sync.dma_start(gidx_sb[:], gIdx[:])

            zrow = cpool.tile([1, 512], F32)
            nc.vector.memset(zrow[:], 0.0)
            identb = cpool.tile([P, P], BF16)
            make_identity(nc, identb[:])
            iota_i = cpool.tile([P, P], mybir.dt.int32)
            nc.gpsimd.iota(iota_i[:], pattern=[[1, P]], base=0, channel_multiplier=0)
            iota_b = cpool.tile([P, P], BF16)
            nc.vector.tensor_copy(iota_b[:], iota_i[:])

            stage1 = cpool.tile([P, NG * DH], BF16)   # G1 own rows (table vals)
            stage2 = cpool.tile([P, NG * DH], BF16)   # G2 own rows

            G1s = dpool.tile([npcp, P], BF16)
            G1f = dpool.tile([C * npcp, P], BF16, addr_space="Shared")
            G2s = dpool.tile([npcp, P], BF16)
            G2f = dpool.tile([C * npcp, P], BF16, addr_space="Shared")
            rg = [list(range(C))]

            # ---- layer 1 dense: stage1 = dinv * (x @ W1) (bf16), to G1s ----
            for g in range(NG):
                ps = ppool.tile([P, DH], F32, tag="pdense", bufs=1, space="PSUM")
                xt_t = wpool.tile([P, P], F32, tag="xt", bufs=3)
                nc.sync.dma_start(xt_t[:], xT[:, g * P : (g + 1) * P])
                nc.tensor.matmul(ps[:], lhsT=xt_t[:], rhs=W1sb[:], start=True, stop=True)
                sl = stage1[:, g * DH : (g + 1) * DH]
                nc.scalar.activation(
                    sl, ps[:], mybir.ActivationFunctionType.Copy,
                    scale=dinv[:, g : g + 1],
                )
                nc.sync.dma_start(G1s[g * P : (g + 1) * P, 0:DH], sl)

            nc.gpsimd.collective_compute(
                "AllGather", mybir.AluOpType.bypass, replica_groups=rg,
                ins=[G1s[:]], outs=[G1f[:]],
            )

            # last nonempty (w, j) per group, for the PSUM stop flag
            last_wj = {}
            for g in range(NG):
                last_wj[g] = None
                for w in range(NWIN):
                    if cfg.cell_nblk(g, w) > 0:
                        last_wj[g] = (w, cfg.cell_nblk(g, w) - 1)

            def run_layer(Gf, layer):
                for o, (g0, gn) in enumerate(oct_groups):
                    # pack group accumulators into bank-sized PSUM tiles
                    psg_of = {}
                    if layer == 1:
                        nbank = -(-gn // 8)
                        banks = [
                            ppool.tile(
                                [P, 512], F32, tag=f"ps1b_{k}", bufs=1,
                                space="PSUM", name=f"psg1_{o}_{k}",
                            )
                            for k in range(nbank)
                        ]
                        for g in range(g0, g0 + gn):
                            i = g - g0
                            psg_of[g] = banks[i // 8][:, (i % 8) * DH : (i % 8 + 1) * DH]
                        for k in range(nbank):
                            # start=True resets the whole PSUM bank on HW, so
                            # zero each bank once and accumulate into slices
                            nc.tensor.matmul(
                                banks[k][:], lhsT=zrow[:, 0:P], rhs=zrow[:],
                                start=True, stop=False, skip_group_check=True,
                            )
                    else:
                        nbank = -(-gn // 4)
                        banks = [
                            ppool.tile(
                                [DH, 512], F32, tag=f"ps2b_{k}", bufs=1,
                                space="PSUM", name=f"psg2_{o}_{k}",
                            )
                            for k in range(nbank)
                        ]
                        for g in range(g0, g0 + gn):
                            i = g - g0
                            psg_of[g] = banks[i // 4][:, (i % 4) * P : (i % 4 + 1) * P]
                        for k in range(nbank):
                            nc.tensor.matmul(
                                banks[k][:], lhsT=zrow[:, 0:DH], rhs=zrow[:],
                                start=True, stop=False, skip_group_check=True,
                            )
                    for w in range(NWIN):
                        b0, nb, c0 = chunk_info[(o, w)]
                        if nb == 0:
                            continue
                        rows = wpool.tile(
                            [P, max_chunk_nb, P], BF16, tag="rows", bufs=2
                        )
                        nc.gpsimd.dma_gather(
                            out_ap=rows[:, 0:nb, :],
                            in_ap=Gf[w * winrows : (w + 1) * winrows, :],
                            idxs_ap=gidx_sb[:, c0 : c0 + nb * 8],
                            num_idxs=nb * P,
                            num_idxs_reg=nb * P,
                            elem_size=P,
                            single_packet=False,
                        )
                        jj = 0
                        for g in range(g0, g0 + gn):
                            cnb = cfg.cell_nblk(g, w)
                            bcol = block_col[(g, w)]
                            psg = psg_of[g]
                            first_w = (w == 0) or all(
                                cfg.cell_nblk(g, w2) == 0 for w2 in range(w)
                            )
                            if first_w:
                                # self-loop + bias injection opens the group
                                only = last_wj[g] is None
                                if layer == 1:
                                    nc.tensor.matmul(
                                        psg, lhsT=identb[:],
                                        rhs=stage1[:, g * DH : (g + 1) * DH],
                                        start=False, stop=False,
                                        skip_group_check=True,
                                    )
                                    nc.tensor.matmul(
                                        psg,
                                        lhsT=sqd[:, g * P : (g + 1) * P],
                                        rhs=b1sb[:],
                                        start=False, stop=only,
                                        skip_group_check=True,
                                    )
                                else:
                                    nc.tensor.matmul(
                                        psg,
                                        lhsT=stage2[:, g * DH : (g + 1) * DH],
                                        rhs=identb[:],
                                        start=False, stop=only,
                                        skip_group_check=True,
                                    )
                            if cnb == 0:
                                continue
                            S = wpool.tile(
                                [P, max_cell_nb, P], BF16, tag="S", bufs=4
                            )
                            nc.vector.tensor_tensor(
                                out=S[:, 0:cnb, :],
                                in0=dst_sb[:, bcol : bcol + cnb]
                                .unsqueeze(2).to_broadcast([P, cnb, P]),
                                in1=iota_b[:].unsqueeze(1).to_broadcast([P, cnb, P]),
                                op=mybir.AluOpType.is_equal,
                            )
                            for j in range(cnb):
                                last = last_wj[g] == (w, j)
                                if layer == 1:
                                    nc.tensor.matmul(
                                        psg, lhsT=S[:, j, :],
                                        rhs=rows[:, jj + j, 0:DH],
                                        start=False, stop=last,
                                        skip_group_check=True,
                                    )
                                else:
                                    nc.tensor.matmul(
                                        psg, lhsT=rows[:, jj + j, 0:DH],
                                        rhs=S[:, j, :],
                                        start=False, stop=last,
                                        skip_group_check=True,
                                    )
                            jj += cnb
                    # finish the octant's groups
                    for g in range(g0, g0 + gn):
                        psg = psg_of[g]
                        if layer == 1:
                            tmp = wpool.tile([P, DH], F32, tag="f1tmp", bufs=3)
                            nc.scalar.activation(
                                tmp[:], psg, mybir.ActivationFunctionType.Relu,
                                scale=dinv[:, g : g + 1],
                            )
                            sl = stage2[:, g * DH : (g + 1) * DH]
                            nc.scalar.activation(
                                sl, tmp[:], mybir.ActivationFunctionType.Copy,
                                scale=dinv[:, g : g + 1],
                            )
                            nc.sync.dma_start(G2s[g * P : (g + 1) * P, 0:DH], sl)
                        else:
                            aT = wpool.tile([DH, P], F32, tag="aT", bufs=3)
                            nc.scalar.activation(
                                aT[:], psg, mybir.ActivationFunctionType.Copy,
                            )
                            zp = ppool.tile(
                                [P, DOUT], F32, tag="zp", bufs=1, space="PSUM"
                            )
                            nc.tensor.matmul(
                                zp[:], lhsT=aT[:], rhs=W2sb[:],
                                start=True, stop=False,
                            )
                            nc.tensor.matmul(
                                zp[:], lhsT=sqd[:, g * P : (g + 1) * P],
                                rhs=b2sb[:], start=False, stop=True,
                            )
                            zs = wpool.tile([P, DOUT], F32, tag="zs", bufs=3)
                            nc.scalar.activation(
                                zs[:], zp[:], mybir.ActivationFunctionType.Copy,
                                scale=dinv[:, g : g + 1],
                            )
                            nc.sync.dma_start(z[g * P : (g + 1) * P, :], zs[:])

            run_layer(G1f, 1)

            nc.gpsimd.collective_compute(
                "AllGather", mybir.AluOpType.bypass, replica_groups=rg,
                ins=[G2s[:]], outs=[G2f[:]],
            )

            run_layer(G2f, 2)

    nc.compile()
    return nc


N_CORES = 8
_NC_CACHE = {}


def _cached_nc(cfg):
    if cfg not in _NC_CACHE:
        _NC_CACHE[cfg] = build_nc(cfg)
    return _NC_CACHE[cfg]


def kernel(x, W1, b1, W2, b2, edge_index):
    x = np.asarray(x)
    n = x.shape[0]
    cfg, sched = build_schedule(np.asarray(edge_index), n, N_CORES)
    in_maps = build_inputs(cfg, x, W1, b1, W2, b2, sched)
    nc = _cached_nc(cfg)
    res = bass_utils.run_bass_kernel_spmd(nc, in_maps, core_ids=list(range(N_CORES)))
    z = np.concatenate(
        [res.results[c]["z"][sched[c]["pos_of_node"]] for c in range(N_CORES)], axis=0
    )
    return z.astype(np.float32)
